# revision 44
# baseline (speedup 1.0000x reference)
"""Trainium2 Bass kernel for nn_DAGLinkPredictor (3-layer GAT + edge decoder).

Sharding: dst-node-sharded GAT across 8 cores. Edges (incl self-loops) are
sorted by dst and grouped into per-core 128-node dst blocks. Per block:
  - dma_gather pulls [h | al_src] rows (bf16) for edge sources from the
    replicated node table T_l (two gathers: src < 32768 and >= 32768, since
    gather indices are int16),
  - a one-hot scatter matrix S (built on VectorE from dst offsets vs an iota
    row) scatter-adds messages into PSUM via TensorE,
  - softmax is denominator-style: out[d] = sum_e exp(lrelu(als+ald)) * h[src]
    / sum_e exp(...), which is exactly segment-softmax without segment-max
    (mathematically identical, and safe here since logits are O(1)).
Node phase (per block): divide, elu, transpose, dense matmul with the next
layer's folded weights -> next table slice. Slices are AllGathered between
layers in 4 row-chunks so the collective overlaps remaining block compute.
Decoder: transpose-gathers of z rows + dense matmuls; trans_bias lookup is
pure input-indexing (types[ls], types[ld] are host-known ints) and is
precomputed into a per-slot bias vector on the host.
"""
import os
import numpy as np
import ml_dtypes

N = 50000
NP = 50176            # padded nodes: 8 * 6272
SLICE = NP // 8       # 6272 nodes per core
NB = SLICE // 128     # 49 blocks per core
E = 800000
EL = 100000
HALF = 32768          # int16 gather index limit
NTYPES = 311
EMB = 16
COMB = 48

# layer configs: (Din, HD, H, D, table_width, fold_col_offset)
# table_width must be a multiple of 128 elems (gather elem_size % 256B == 0)
LCFG = [
    (48, 256, 4, 64, 384, 256),
    (256, 256, 2, 128, 384, 256),
    (256, 128, 1, 128, 256, 128),
]
TDEC_W = 128          # decode table row elems (bf16, 256B)
DEC_TILE = 512        # decode matmul tile
DEC_GATHER = int(os.environ.get("K_DECG", "512"))   # decode gather super-tile
NCHUNK = int(os.environ.get("K_NCHUNK", "7"))       # AllGather chunks/layer
CHUNK_BLOCKS = [NB // NCHUNK] * (NCHUNK - 1) + [NB - (NB // NCHUNK) * (NCHUNK - 1)]

bf16 = ml_dtypes.bfloat16

# Chunked-AllGather table layout: T rows are [chunk0 of all cores |
# chunk1 of all cores | ...] so each chunk's collective output is a
# contiguous row range. trow() maps node id -> permuted T row.
CHUNK_R = [cb * 128 for cb in CHUNK_BLOCKS]            # rows per chunk
CHUNK_R0 = [sum(CHUNK_R[:q]) for q in range(NCHUNK)]   # slice-local starts
CHUNK_O = [8 * sum(CHUNK_R[:q]) for q in range(NCHUNK)]  # T row starts


def _trow_map():
    m = np.empty(NP, np.int64)
    for q in range(NCHUNK):
        for c in range(8):
            s0 = c * SLICE + CHUNK_R0[q]
            t0 = CHUNK_O[q] + c * CHUNK_R[q]
            m[s0:s0 + CHUNK_R[q]] = np.arange(t0, t0 + CHUNK_R[q])
    return m


TROW = _trow_map()


def _wrap_idx16(vals):
    """int16 index array [16, n/16] with element (p, c) = vals[c*16+p].
    (On device this is replicated across the 8 gpsimd core groups.)"""
    n = len(vals)
    assert n % 16 == 0
    v = np.asarray(vals, np.int64)
    assert v.min() >= 0 and v.max() < 32768
    return v.reshape(n // 16, 16).T.astype(np.int16)


def _slotmajor(vals, fill, dtype):
    """[128, n/128] array with element (p, c) = vals[c*128+p]."""
    n = len(vals)
    assert n % 128 == 0
    return np.asarray(vals, np.float64).reshape(n // 128, 128).T.astype(dtype)


def prep(x, edge_index, edge_label_index, emb, W1, a_src1, a_dst1, b1,
         W2, a_src2, a_dst2, b2, W3, a_src3, a_dst3, b3,
         Wl1, bl1, Wl2, bl2, trans_bias):
    """Host-side (integer/index + weight-layout) preprocessing."""
    types = x[:, 0].astype(np.int64)

    # --- weight folds: RHS_l = [W_l | W_l@a_src per head | W_l@a_dst] ---
    def fold(W, a_s, a_d, H, D):
        cols_s = np.stack([W[:, h * D:(h + 1) * D] @ a_s[h] for h in range(H)], 1)
        cols_d = np.stack([W[:, h * D:(h + 1) * D] @ a_d[h] for h in range(H)], 1)
        return np.concatenate([W, cols_s, cols_d], 1).astype(np.float32)
    RHS = [fold(W1, a_src1, a_dst1, 4, 64),
           fold(W2, a_src2, a_dst2, 2, 128),
           fold(W3, a_src3, a_dst3, 1, 128)]

    # --- x_comb = [emb[type] | x[:,1:]] transposed, per core slice ---
    emb_f = np.asarray(emb, np.float32)
    x_comb = np.concatenate([emb_f[types], np.asarray(x[:, 1:], np.float32)], 1)

    # --- edges: add self loops, sort by dst, bucket per core / block ---
    loops = np.arange(N, dtype=np.int64)
    src = np.concatenate([edge_index[0].astype(np.int64), loops])
    dst = np.concatenate([edge_index[1].astype(np.int64), loops])
    order = np.argsort(dst, kind="stable")
    src, dst = src[order], dst[order]

    blk = dst // 128          # global block id (0..391)
    tsrc = TROW[src]          # source row in the permuted table layout
    per = [[[None, None] for _ in range(NB)] for _ in range(8)]
    for c in range(8):
        for b in range(NB):
            gb = c * NB + b
            m = blk == gb
            s, d = tsrc[m], dst[m]
            lo = s < HALF
            per[c][b][0] = (s[lo], d[lo])
            per[c][b][1] = (s[~lo] - HALF, d[~lo])
    # chunk counts, shared across cores (same kernel structure)
    CA = [max(1, max((len(per[c][b][0][0]) + 127) // 128 for c in range(8)))
          for b in range(NB)]
    CB = [max((len(per[c][b][1][0]) + 127) // 128 for c in range(8))
          for b in range(NB)]

    idxA, idxB, doff = [], [], []
    for c in range(8):
        la, lb, lo = [], [], []
        for b in range(NB):
            for half, (cnt, acc) in (((0), (CA[b], la)), ((1), (CB[b], lb))):
                s, d = per[c][b][half]
                ns = cnt * 128
                sp = np.zeros(ns, np.int64)
                sp[:len(s)] = s
                acc.append(sp)
                off = np.full(ns, 255, np.int64)     # 255 => padded slot
                off[:len(d)] = d[:len(d)] - (c * SLICE + b * 128)
                lo.append(off)
        z = bool(os.environ.get("K_IDX0"))    # timing experiment only
        idxA.append(_wrap_idx16(np.concatenate(la) * 0 if z else np.concatenate(la)))
        idxB.append(_wrap_idx16(np.concatenate(lb) * 0 if z else np.concatenate(lb)))
        doff.append(_slotmajor(np.concatenate(lo), 255, np.float32))

    # --- label edges: 4 groups by (ls-half, ld-half), padded per group ---
    ls = edge_label_index[0].astype(np.int64)
    ld_ = edge_label_index[1].astype(np.int64)
    tls_r = TROW[ls]
    tld_r = TROW[ld_]
    elpc = (EL + 7) // 8                      # 12500 label edges per core
    groups_sz = np.zeros((8, 4), np.int64)
    per_dec = [[None] * 4 for _ in range(8)]
    for c in range(8):
        lo_, hi_ = c * elpc, min((c + 1) * elpc, EL)
        eidx = np.arange(lo_, hi_)
        g = (tls_r[eidx] >= HALF).astype(np.int64) * 2 + (tld_r[eidx] >= HALF)
        for gi in range(4):
            per_dec[c][gi] = eidx[g == gi]
            groups_sz[c, gi] = len(per_dec[c][gi])
    GSZ = [int(-(-groups_sz[:, gi].max() // DEC_TILE) * DEC_TILE)
           for gi in range(4)]
    SL = sum(GSZ)
    tb = np.asarray(trans_bias, np.float32)
    bl2v = float(np.asarray(bl2).reshape(-1)[0])
    lsw, ldw, dbias, slotmap = [], [], [], []
    for c in range(8):
        a_ls = np.zeros(SL, np.int64)
        a_ld = np.zeros(SL, np.int64)
        a_bias = np.zeros(SL, np.float64)
        smap = np.full(SL, -1, np.int64)
        pos = 0
        for gi in range(4):
            e = per_dec[c][gi]
            n = len(e)
            a_ls[pos:pos + n] = tls_r[e] - (HALF if gi >= 2 else 0)
            a_ld[pos:pos + n] = tld_r[e] - (HALF if gi % 2 else 0)
            a_bias[pos:pos + n] = tb[types[ls[e]], types[ld_[e]]]
            smap[pos:pos + n] = e
            pos += GSZ[gi]
        lsw.append(_wrap_idx16(a_ls))
        ldw.append(_wrap_idx16(a_ld))
        dbias.append(_slotmajor(a_bias + bl2v, 0, np.float32))
        slotmap.append(smap)

    iota128 = np.tile(np.arange(128, dtype=np.float32)[None, :], (128, 1))
    ident = np.eye(128, dtype=np.float32)

    in_maps = []
    for c in range(8):
        xcT = np.zeros((COMB, SLICE), np.float32)
        n0 = c * SLICE
        n1 = min((c + 1) * SLICE, N)
        if n1 > n0:
            xcT[:, :n1 - n0] = x_comb[n0:n1].T
        in_maps.append(dict(
            xcT=xcT,
            RHS1=RHS[0], RHS2=RHS[1], RHS3=RHS[2],
            idxA=idxA[c], idxB=idxB[c], doff=doff[c],
            ls_idx=lsw[c], ld_idx=ldw[c], dec_bias=dbias[c],
            iota128=iota128, ident=ident,
            Wl1a=Wl1[:128].astype(bf16), Wl1b=Wl1[128:].astype(bf16),
            Wl2=Wl2.astype(bf16), bl1=bl1.reshape(64, 1).astype(np.float32),
        ))
    cfg = dict(CA=CA, CB=CB, GSZ=GSZ, SL=SL,
               SA=sum(CA) * 128, SB=sum(CB) * 128,
               ST=(sum(CA) + sum(CB)) * 128,
               slotmap=slotmap)
    return in_maps, cfg


# ---------------------------------------------------------------- golden ---
def golden(in_maps, cfg):
    """numpy mirror of the device algorithm (fp32; layout-accurate)."""
    CA, CB = cfg["CA"], cfg["CB"]
    T = None
    for li, (Din, HD, H, D, W_, OFF) in enumerate(LCFG):
        Tn = np.zeros((NP, W_), np.float32)
        slices = []
        for c in range(8):
            im = in_maps[c]
            if li == 0:
                hrow = im["xcT"].T @ im["RHS1"]
            else:
                hrow = PREV[c] @ im[f"RHS{li + 1}"]
            sl = np.zeros((SLICE, W_), np.float32)
            sl[:, :hrow.shape[1]] = hrow
            slices.append(sl)
            Tn[c * SLICE:(c + 1) * SLICE] = sl
        Tp = np.zeros_like(Tn)
        Tp[TROW] = Tn          # permuted chunk-major table layout
        T = Tp.astype(bf16).astype(np.float32)
        slices = [s.astype(bf16).astype(np.float32) for s in slices]
        # edge phase
        PREV = []
        for c in range(8):
            im = in_maps[c]
            ia = _unwrap(im["idxA"], cfg["SA"])
            ib = _unwrap(im["idxB"], cfg["SB"])
            dof = im["doff"].T.reshape(-1)  # slot order
            xl = np.zeros((SLICE, HD), np.float32)
            pa = pb = pt = 0
            for b in range(NB):
                sA, sB = CA[b] * 128, CB[b] * 128
                gidx = np.concatenate([ia[pa:pa + sA],
                                       ib[pb:pb + sB] + HALF])
                pa += sA; pb += sB
                nsl = sA + sB
                G = T[gidx, :]                     # [nsl, W_]
                off = dof[pt:pt + nsl]
                pt += nsl
                S = (off[:, None] == np.arange(128)[None, :]).astype(np.float32)
                als = G[:, OFF:OFF + H]
                # ald per edge via one-hot lookup against the block's rows
                ald_blk = slices[c][b * 128:(b + 1) * 128, OFF + H:OFF + 2 * H]
                ald = S @ ald_blk.astype(bf16).astype(np.float32)
                lg = als + ald
                e = np.maximum(np.exp(lg), np.exp(0.2 * lg))
                e = e.astype(bf16).astype(np.float32)
                msg = (G[:, :HD].reshape(nsl, H, D) * e[:, :, None]
                       ).reshape(nsl, HD).astype(bf16).astype(np.float32)
                num = S.T @ msg                    # [128, HD]
                den = S.T @ e                      # [128, H]
                r = 1.0 / (den + 1e-16)
                xb = (num.reshape(128, H, D) * r[:, :, None]).reshape(128, HD)
                if li < 2:
                    xb = np.maximum(xb, 0) - 1 + np.exp(np.minimum(xb, 0))
                xl[b * 128:(b + 1) * 128] = xb
            PREV.append(xl)
    # decode
    TDn = np.zeros((NP, TDEC_W), np.float32)
    for c in range(8):
        TDn[c * SLICE:(c + 1) * SLICE] = PREV[c].astype(bf16)
    TD = np.zeros_like(TDn)
    TD[TROW] = TDn             # permuted chunk-major layout
    TD = TD.astype(bf16).astype(np.float32)
    scores = []
    for c in range(8):
        im = in_maps[c]
        lsv = _unwrap(im["ls_idx"], cfg["SL"])
        ldv = _unwrap(im["ld_idx"], cfg["SL"])
        base_ls = np.zeros(cfg["SL"], np.int64)
        base_ld = np.zeros(cfg["SL"], np.int64)
        pos = 0
        for gi in range(4):
            base_ls[pos:pos + cfg["GSZ"][gi]] = HALF if gi >= 2 else 0
            base_ld[pos:pos + cfg["GSZ"][gi]] = HALF if gi % 2 else 0
            pos += cfg["GSZ"][gi]
        zl = TD[lsv + base_ls]
        zr = TD[ldv + base_ld]
        W1a = im["Wl1a"].astype(np.float32)
        W1b = im["Wl1b"].astype(np.float32)
        h = np.maximum(zl @ W1a + zr @ W1b + im["bl1"].T, 0).astype(bf16).astype(np.float32)
        base = h @ im["Wl2"].astype(np.float32)
        bias = im["dec_bias"].T.reshape(-1)
        scores.append(base[:, 0] + bias)
    out = np.zeros((EL, 1), np.float32)
    for c in range(8):
        m = cfg["slotmap"][c] >= 0
        out[cfg["slotmap"][c][m], 0] = scores[c][m]
    return out


def _unwrap(w, n):
    return w[:16, :].T.reshape(-1)[:n].astype(np.int64)


# ----------------------------------------------------------------- device ---
def build(cfg):
    import concourse.bacc as bacc
    import concourse.mybir as mybir
    from concourse.tile import TileContext
    dt = mybir.dt
    F = mybir.ActivationFunctionType
    A = mybir.AluOpType
    CA, CB, SL = cfg["CA"], cfg["CB"], cfg["SL"]
    SA, SB, ST = cfg["SA"], cfg["SB"], cfg["ST"]

    GO = bool(os.environ.get("K_GATHERONLY"))   # timing experiment only
    scratch = int(os.environ.get("K_SCRATCH", "32768"))
    nc = bacc.Bacc(num_devices=8, dynamic_dma_scratch_size=scratch)
    # max 128-chunks per dma_gather call. The HW SWDGE descriptor ring
    # limits per-call size; 6 (768 idx) is proven stable at 32KB scratch.
    GMAX = int(os.environ.get("K_GMAX", "6"))

    def gat(out_ap, in_ap, idx_tile, col0, nchunk, elem, **kw):
        for s0 in range(0, nchunk, GMAX):
            s1 = min(s0 + GMAX, nchunk)
            nc.gpsimd.dma_gather(
                out_ap[:, s0:s1, :], in_ap,
                idx_tile[:, col0 + s0 * 8: col0 + s1 * 8],
                (s1 - s0) * 128, (s1 - s0) * 128, elem, **kw)

    inp = {}
    for name, shape, d in [
        ("xcT", [COMB, SLICE], dt.float32),
        ("RHS1", [48, 264], dt.float32),
        ("RHS2", [256, 260], dt.float32),
        ("RHS3", [256, 130], dt.float32),
        ("idxA", [16, SA // 16], dt.int16),
        ("idxB", [16, SB // 16], dt.int16),
        ("doff", [128, ST // 128], dt.float32),
        ("ls_idx", [16, SL // 16], dt.int16),
        ("ld_idx", [16, SL // 16], dt.int16),
        ("dec_bias", [128, SL // 128], dt.float32),
        ("iota128", [128, 128], dt.float32),
        ("ident", [128, 128], dt.float32),
        ("Wl1a", [128, 64], dt.bfloat16),
        ("Wl1b", [128, 64], dt.bfloat16),
        ("Wl2", [64, 1], dt.bfloat16),
        ("bl1", [64, 1], dt.float32),
    ]:
        inp[name] = nc.dram_tensor(name, shape, d, kind="ExternalInput")
    score_out = nc.dram_tensor("score", [SL, 1], dt.float32, kind="ExternalOutput")

    sl_t = [nc.dram_tensor(f"slice{l}", [SLICE, LCFG[l][4]], dt.bfloat16,
                           kind="Internal") for l in range(3)]
    sl_d = nc.dram_tensor("sliceD", [SLICE, TDEC_W], dt.bfloat16, kind="Internal")
    T_t = [nc.dram_tensor(f"T{l}", [NP, LCFG[l][4]], dt.bfloat16,
                          kind="Internal", addr_space="Shared") for l in range(3)]
    T_d = nc.dram_tensor("TD", [NP, TDEC_W], dt.bfloat16,
                         kind="Internal", addr_space="Shared")

    def chunked_allgather(sl_tensor, T_tensor, width, q):
        if os.environ.get("K_SKIPCOLL"):      # timing experiment only
            return
        r0, r1 = CHUNK_R0[q], CHUNK_R0[q] + CHUNK_R[q]
        o0 = CHUNK_O[q]
        nc.gpsimd.collective_compute(
            "AllGather", mybir.AluOpType.bypass,
            ins=[sl_tensor[r0:r1, :]],
            outs=[T_tensor[o0:o0 + 8 * CHUNK_R[q], :]],
            replica_groups=[list(range(8))])

    with TileContext(nc, num_cores=8) as tc:
        with tc.tile_pool(name="const", bufs=1) as cpool, \
             tc.tile_pool(name="work", bufs=2) as wpool, \
             tc.tile_pool(name="psum", bufs=2, space="PSUM") as ppool, \
             tc.tile_pool(name="psum1", bufs=1, space="PSUM") as ppool1:
            # ---- resident constants / indices ----
            def load(name, shape, d):
                t = cpool.tile(shape, d, tag=name)
                nc.sync.dma_start(t[:], inp[name][:])
                return t

            def load_idx(name, cols):
                """int16 idx [16, cols] replicated into [128, cols]."""
                t = cpool.tile([128, cols], dt.int16, tag=name)
                for g in range(8):
                    nc.sync.dma_start(t[16 * g:16 * g + 16, :], inp[name][:])
                return t
            idxA = load_idx("idxA", SA // 16)
            idxB = load_idx("idxB", SB // 16)
            doff = load("doff", [128, ST // 128], dt.float32)
            # per-layer resident [128, NB, H] dst-fold values (tiny): written
            # by the prologue / node phase, read by the next layer's ald matmul
            aldt = []
            for l in range(3):
                ald_l = cpool.tile([128, NB, LCFG[l][2]], dt.bfloat16,
                                   tag=f"ald{l}", name=f"ald{l}")
                aldt.append(ald_l)
            iota = load("iota128", [128, 128], dt.float32)
            ident = load("ident", [128, 128], dt.float32)
            identb = cpool.tile([128, 128], dt.bfloat16, tag="identb")
            nc.vector.tensor_copy(identb[:], ident[:])
            RHSs = [load("RHS1", [48, 264], dt.float32)]
            for l, w in ((2, 260), (3, 130)):
                t = cpool.tile([128, 2, w], dt.float32, tag=f"RHS{l}")
                nc.sync.dma_start(
                    t[:], inp[f"RHS{l}"][:].rearrange("(k p) w -> p k w", p=128))
                RHSs.append(t)

            # ---- prologue: T1 rows = x_comb @ RHS1 ----
            W0 = LCFG[0][4]
            done_blocks = 0
            for q in range(NCHUNK):
                for b in range(done_blocks, done_blocks + CHUNK_BLOCKS[q]):
                    if GO:
                        continue
                    xc = wpool.tile([COMB, 128], dt.float32, tag="xc")
                    nc.sync.dma_start(xc[:], inp["xcT"][:, b * 128:(b + 1) * 128])
                    pn = ppool1.tile([128, 264], dt.float32, tag="pn")
                    nc.tensor.matmul(pn[:, 0:264],
                                     xc[:],
                                     RHSs[0][:], start=True, stop=True)
                    row = wpool.tile([128, W0], dt.bfloat16, tag="row")
                    nc.vector.tensor_copy(row[:, 0:264], pn[:, 0:264])
                    nc.vector.memset(row[:, 264:W0], 0)
                    nc.vector.tensor_copy(aldt[0][:, b, :], pn[:, 260:264])
                    nc.sync.dma_start(sl_t[0][b * 128:(b + 1) * 128, :],
                                      row[:, 0:W0])
                done_blocks += CHUNK_BLOCKS[q]
                chunked_allgather(sl_t[0], T_t[0], W0, q)

            # ---- three GAT layers ----
            for li, (Din, HD, H, D, W_, OFF) in enumerate(LCFG):
                RW = HD + H
                ALD0 = OFF + H - (W_ - 128)   # ald offset inside GD window
                pa = pb = pt_ = 0
                done_blocks = 0
                for q in range(NCHUNK):
                  for b in range(done_blocks, done_blocks + CHUNK_BLOCKS[q]):
                    cA, cB = CA[b], CB[b]
                    C = cA + cB
                    G = wpool.tile([128, C, W_], dt.bfloat16, tag="G")
                    if os.environ.get("K_GELEM128"):  # timing experiment only
                        G128 = wpool.tile([128, C, 128], dt.bfloat16, tag="G128")
                        gat(G128[:, :cA, :].rearrange("p c e -> p c e"),
                            T_t[li][:, 0:128], idxA[:], pa // 16, cA,
                            128, elem_step=W_)
                        if cB:
                            gat(G128[:, cA:C, :].rearrange("p c e -> p c e"),
                                T_t[li][HALF:, 0:128], idxB[:], pb // 16, cB,
                                128, elem_step=W_)
                    else:
                        gat(G, T_t[li][:, :], idxA[:], pa // 16, cA, W_)
                        if cB:
                            gat(G[:, cA:C, :].rearrange("p c e -> p c e"),
                                T_t[li][HALF:, :], idxB[:], pb // 16, cB, W_)
                    if GO:
                        pa += cA * 128
                        pb += cB * 128
                        pt_ += C * 128
                        continue
                    S = wpool.tile([128, C, 128], dt.bfloat16, tag="S")
                    nc.vector.tensor_tensor(
                        S[:],
                        doff[:, pt_ // 128: pt_ // 128 + C].unsqueeze(-1)
                            .broadcast_to([128, C, 128]),
                        iota[:].unsqueeze(1).broadcast_to([128, C, 128]),
                        A.is_equal)
                    # ald per edge slot: transpose each S chunk on TensorE,
                    # then one-hot matmul against the block's ald rows
                    ST = wpool.tile([128, C, 128], dt.bfloat16, tag="ST")
                    pald = ppool1.tile([128, C, H], dt.float32, tag="pald")
                    for ch in range(C):
                        ptp = ppool1.tile([128, 128], dt.bfloat16, tag="ptb")
                        nc.tensor.transpose(ptp[:], S[:, ch, :], identb[:])
                        nc.vector.tensor_copy(ST[:, ch, :], ptp[:])
                        nc.tensor.matmul(pald[:, ch, :], ST[:, ch, :],
                                         aldt[li][:, b, :],
                                         start=True, stop=True)
                    lg = wpool.tile([128, C, H], dt.float32, tag="lg")
                    nc.vector.tensor_tensor(
                        lg[:], G[:, :, OFF:OFF + H], pald[:], A.add)
                    # e = exp(leaky_relu(lg)) = max(exp(lg), exp(0.2*lg))
                    e1 = wpool.tile([128, C, H], dt.float32, tag="e1")
                    nc.scalar.activation(e1[:], lg[:], F.Exp)
                    e2 = wpool.tile([128, C, H], dt.float32, tag="e2")
                    nc.scalar.activation(e2[:], lg[:], F.Exp, scale=0.2)
                    RT = wpool.tile([128, C, RW], dt.bfloat16, tag="RT")
                    nc.vector.tensor_tensor(RT[:, :, HD:HD + H], e1[:], e2[:],
                                            A.max)
                    nc.vector.tensor_tensor(
                        RT[:, :, 0:HD].rearrange("p c (h d) -> p c h d", h=H),
                        G[:, :, 0:HD].rearrange("p c (h d) -> p c h d", h=H),
                        RT[:, :, HD:HD + H].unsqueeze(-1)
                            .broadcast_to([128, C, H, D]),
                        A.mult)
                    pe = ppool.tile([128, RW], dt.float32, tag="pe")
                    for ch in range(C):
                        nc.tensor.matmul(pe[:, 0:RW], S[:, ch, :], RT[:, ch, :],
                                         start=(ch == 0), stop=(ch == C - 1))
                    pa += cA * 128
                    pb += cB * 128
                    pt_ += C * 128
                    # ---- finalize + node phase ----
                    den = wpool.tile([128, H], dt.float32, tag="den")
                    nc.vector.tensor_scalar_add(den[:], pe[:, HD:HD + H], 1e-16)
                    rec = wpool.tile([128, H], dt.float32, tag="rec")
                    nc.vector.reciprocal(rec[:], den[:])
                    xo = wpool.tile([128, HD], dt.float32, tag="xo")
                    nc.vector.tensor_tensor(
                        xo[:].rearrange("p (h d) -> p h d", h=H),
                        pe[:, 0:HD].rearrange("p (h d) -> p h d", h=H),
                        rec[:].unsqueeze(-1).broadcast_to([128, H, D]),
                        A.mult)
                    if li < 2:
                        m = wpool.tile([128, HD], dt.float32, tag="melu")
                        nc.vector.tensor_scalar_min(m[:], xo[:], 0.0)
                        e3 = wpool.tile([128, HD], dt.float32, tag="e3")
                        nc.scalar.activation(e3[:], m[:], F.Exp)
                        # xo = (max(xo,0) - 1) + exp(min(xo,0))
                        nc.vector.tensor_scalar(xo[:], xo[:], 0.0, -1.0,
                                                A.max, A.add)
                        nc.vector.tensor_tensor(xo[:], xo[:], e3[:], A.add)
                        # node phase: T_{l+1} row = [x @ W' | folds]
                        NDin, NHD, NH, ND, NW_, _ = LCFG[li + 1]
                        NW = NHD + 2 * NH
                        xT = wpool.tile([128, 2, 128], dt.float32, tag="xT")
                        for kc in range(2):
                            ptp = ppool1.tile([128, 128], dt.float32, tag="pt")
                            nc.tensor.transpose(
                                ptp[:], xo[:, kc * 128:(kc + 1) * 128], ident[:])
                            nc.vector.tensor_copy(xT[:, kc, :], ptp[:])
                        pn = ppool1.tile([128, 264], dt.float32, tag="pn")
                        for kc in range(2):
                            nc.tensor.matmul(pn[:, 0:NW], xT[:, kc, :],
                                             RHSs[li + 1][:, kc, :],
                                             start=(kc == 0), stop=(kc == 1))
                        row = wpool.tile([128, NW_], dt.bfloat16, tag="nrow")
                        nc.vector.tensor_copy(row[:, 0:NW], pn[:, 0:NW])
                        if NW < NW_:
                            nc.vector.memset(row[:, NW:NW_], 0)
                        nc.vector.tensor_copy(
                            aldt[li + 1][:, b, :],
                            pn[:, NHD + NH:NHD + 2 * NH])
                        nc.sync.dma_start(
                            sl_t[li + 1][b * 128:(b + 1) * 128, :],
                            row[:, 0:NW_])
                    else:
                        rowd = wpool.tile([128, TDEC_W], dt.bfloat16, tag="rowd")
                        nc.vector.tensor_copy(rowd[:], xo[:])
                        nc.sync.dma_start(
                            sl_d[b * 128:(b + 1) * 128, :], rowd[:])
                  done_blocks += CHUNK_BLOCKS[q]
                  if li < 2:
                      chunked_allgather(sl_t[li + 1], T_t[li + 1],
                                        LCFG[li + 1][4], q)
                  else:
                      chunked_allgather(sl_d, T_d, TDEC_W, q)

            # ---- decoder ----
            lsi = load_idx("ls_idx", SL // 16)
            ldi = load_idx("ld_idx", SL // 16)
            dbias = load("dec_bias", [128, SL // 128], dt.float32)
            W1a = load("Wl1a", [128, 64], dt.bfloat16)
            W1b = load("Wl1b", [128, 64], dt.bfloat16)
            W2d = load("Wl2", [64, 1], dt.bfloat16)
            bl1 = load("bl1", [64, 1], dt.float32)
            score_sb = cpool.tile([128, SL // 128], dt.float32, tag="score")
            if GO:
                nc.vector.memset(score_sb[:], 0)
            pos = 0
            for gi in range(4):
                gls, gld = (HALF if gi >= 2 else 0), (HALF if gi % 2 else 0)
                t0 = pos
                while t0 < pos + cfg["GSZ"][gi]:
                    nidx = min(DEC_GATHER, pos + cfg["GSZ"][gi] - t0)
                    zl = wpool.tile([128, 1, nidx], dt.bfloat16, tag=f"zl{nidx}")
                    nc.gpsimd.dma_gather(
                        zl[:], T_d[gls:, :], lsi[:, t0 // 16:(t0 + nidx) // 16],
                        nidx, nidx, TDEC_W, transpose=True)
                    zr = wpool.tile([128, 1, nidx], dt.bfloat16, tag=f"zr{nidx}")
                    nc.gpsimd.dma_gather(
                        zr[:], T_d[gld:, :], ldi[:, t0 // 16:(t0 + nidx) // 16],
                        nidx, nidx, TDEC_W, transpose=True)
                    for s0 in range(0, nidx, DEC_TILE):
                        if GO:
                            continue
                        ph = ppool1.tile([64, DEC_TILE], dt.float32, tag="ph")
                        nc.tensor.matmul(ph[:], W1a[:], zl[:, 0, s0:s0 + DEC_TILE],
                                         start=True, stop=False)
                        nc.tensor.matmul(ph[:], W1b[:], zr[:, 0, s0:s0 + DEC_TILE],
                                         start=False, stop=True)
                        hd = wpool.tile([64, DEC_TILE], dt.bfloat16, tag="hd")
                        nc.scalar.activation(hd[:], ph[:], F.Relu, bias=bl1[:])
                        for sub in range(DEC_TILE // 128):
                            pss = ppool1.tile([128, 1], dt.float32, tag="pss")
                            nc.tensor.matmul(
                                pss[:], hd[:, sub * 128:(sub + 1) * 128],
                                W2d[:], start=True, stop=True)
                            col = (t0 + s0) // 128 + sub
                            nc.vector.tensor_tensor(
                                score_sb[:, col:col + 1], pss[:],
                                dbias[:, col:col + 1], A.add)
                    t0 += nidx
                pos += cfg["GSZ"][gi]
            nc.sync.dma_start(
                score_out[:].rearrange("(c p) o -> p (c o)", p=128), score_sb[:])
    nc.finalize()
    return nc


def kernel(**inputs):
    inputs = {k: np.asarray(v) for k, v in inputs.items()}
    in_maps, cfg = prep(**inputs)
    nc = build(cfg)
    from concourse.bass_utils import run_bass_kernel_spmd
    res = run_bass_kernel_spmd(nc, in_maps, core_ids=list(range(8)))
    out = np.zeros((EL, 1), np.float32)
    for c in range(8):
        sc = res.results[c]["score"][:, 0]
        m = cfg["slotmap"][c] >= 0
        out[cfg["slotmap"][c][m], 0] = sc[m]
    return out


# revision 57
# speedup vs baseline: 1.5203x; 1.5203x over previous
"""Trainium2 Bass kernel for nn_DAGLinkPredictor (3-layer GAT + edge decoder).

Sharding: dst-node-sharded GAT across 8 cores. Edges (incl self-loops) are
sorted by dst and grouped into per-core 128-node dst blocks. Per block:
  - dma_gather pulls [h | al_src] rows (bf16) for edge sources from the
    replicated node table T_l (two gathers: src < 32768 and >= 32768, since
    gather indices are int16),
  - a one-hot scatter matrix S (built on VectorE from dst offsets vs an iota
    row) scatter-adds messages into PSUM via TensorE,
  - softmax is denominator-style: out[d] = sum_e exp(lrelu(als+ald)) * h[src]
    / sum_e exp(...), which is exactly segment-softmax without segment-max
    (mathematically identical, and safe here since logits are O(1)).
Node phase (per block): divide, elu, transpose, dense matmul with the next
layer's folded weights -> next table slice. Slices are AllGathered between
layers in 4 row-chunks so the collective overlaps remaining block compute.
Decoder: transpose-gathers of z rows + dense matmuls; trans_bias lookup is
pure input-indexing (types[ls], types[ld] are host-known ints) and is
precomputed into a per-slot bias vector on the host.
"""
import os
import numpy as np
import ml_dtypes

N = 50000
NP = 50176            # padded nodes: 8 * 6272
SLICE = NP // 8       # 6272 nodes per core
NB = SLICE // 128     # 49 blocks per core
E = 800000
EL = 100000
HALF = 32768          # int16 gather index limit
NTYPES = 311
EMB = 16
COMB = 48

# layer configs: (Din, HD, H, D, table_width, fold_col_offset)
# table_width must be a multiple of 128 elems (gather elem_size % 256B == 0)
LCFG = [
    (48, 256, 4, 64, 384, 256),
    (256, 256, 2, 128, 384, 256),
    (256, 128, 1, 128, 256, 128),
]
TDEC_W = 128          # decode table row elems (bf16, 256B)
DEC_TILE = 512        # decode matmul tile
DEC_GATHER = int(os.environ.get("K_DECG", "512"))   # decode gather super-tile
NCHUNK = int(os.environ.get("K_NCHUNK", "7"))       # AllGather chunks/layer
CHUNK_BLOCKS = [NB // NCHUNK] * (NCHUNK - 1) + [NB - (NB // NCHUNK) * (NCHUNK - 1)]

bf16 = ml_dtypes.bfloat16

# Chunked-AllGather table layout: T rows are [chunk0 of all cores |
# chunk1 of all cores | ...] so each chunk's collective output is a
# contiguous row range. trow() maps node id -> permuted T row.
CHUNK_R = [cb * 128 for cb in CHUNK_BLOCKS]            # rows per chunk
CHUNK_R0 = [sum(CHUNK_R[:q]) for q in range(NCHUNK)]   # slice-local starts
CHUNK_O = [8 * sum(CHUNK_R[:q]) for q in range(NCHUNK)]  # T row starts


def _trow_map():
    m = np.empty(NP, np.int64)
    for q in range(NCHUNK):
        for c in range(8):
            s0 = c * SLICE + CHUNK_R0[q]
            t0 = CHUNK_O[q] + c * CHUNK_R[q]
            m[s0:s0 + CHUNK_R[q]] = np.arange(t0, t0 + CHUNK_R[q])
    return m


TROW = _trow_map()


def _wrap_idx16(vals):
    """int16 index array [16, n/16] with element (p, c) = vals[c*16+p].
    (On device this is replicated across the 8 gpsimd core groups.)"""
    n = len(vals)
    assert n % 16 == 0
    v = np.asarray(vals, np.int64)
    assert v.min() >= 0 and v.max() < 32768
    return v.reshape(n // 16, 16).T.astype(np.int16)


def _slotmajor(vals, fill, dtype):
    """[128, n/128] array with element (p, c) = vals[c*128+p]."""
    n = len(vals)
    assert n % 128 == 0
    return np.asarray(vals, np.float64).reshape(n // 128, 128).T.astype(dtype)


def prep(x, edge_index, edge_label_index, emb, W1, a_src1, a_dst1, b1,
         W2, a_src2, a_dst2, b2, W3, a_src3, a_dst3, b3,
         Wl1, bl1, Wl2, bl2, trans_bias):
    """Host-side (integer/index + weight-layout) preprocessing."""
    types = x[:, 0].astype(np.int64)

    # --- weight folds: RHS_l = [W_l | W_l@a_src per head | W_l@a_dst] ---
    def fold(W, a_s, a_d, H, D):
        cols_s = np.stack([W[:, h * D:(h + 1) * D] @ a_s[h] for h in range(H)], 1)
        cols_d = np.stack([W[:, h * D:(h + 1) * D] @ a_d[h] for h in range(H)], 1)
        return np.concatenate([W, cols_s, cols_d], 1).astype(np.float32)
    RHS = [fold(W1, a_src1, a_dst1, 4, 64),
           fold(W2, a_src2, a_dst2, 2, 128),
           fold(W3, a_src3, a_dst3, 1, 128)]

    # --- x_comb = [emb[type] | x[:,1:]] transposed, per core slice ---
    emb_f = np.asarray(emb, np.float32)
    x_comb = np.concatenate([emb_f[types], np.asarray(x[:, 1:], np.float32)], 1)

    # --- edges: add self loops, sort by dst, bucket per core / block ---
    loops = np.arange(N, dtype=np.int64)
    src = np.concatenate([edge_index[0].astype(np.int64), loops])
    dst = np.concatenate([edge_index[1].astype(np.int64), loops])
    order = np.argsort(dst, kind="stable")
    src, dst = src[order], dst[order]

    blk = dst // 128          # global block id (0..391)
    tsrc = TROW[src]          # source row in the permuted table layout
    per = [[[None, None] for _ in range(NB)] for _ in range(8)]
    for c in range(8):
        for b in range(NB):
            gb = c * NB + b
            m = blk == gb
            s, d = tsrc[m], dst[m]
            lo = s < HALF
            per[c][b][0] = (s[lo], d[lo])
            per[c][b][1] = (s[~lo] - HALF, d[~lo])
    # chunk counts, shared across cores (same kernel structure)
    CA = [max(1, max((len(per[c][b][0][0]) + 127) // 128 for c in range(8)))
          for b in range(NB)]
    CB = [max((len(per[c][b][1][0]) + 127) // 128 for c in range(8))
          for b in range(NB)]

    idxA, idxB, doff = [], [], []
    for c in range(8):
        la, lb, lo = [], [], []
        for b in range(NB):
            for half, (cnt, acc) in (((0), (CA[b], la)), ((1), (CB[b], lb))):
                s, d = per[c][b][half]
                ns = cnt * 128
                sp = np.zeros(ns, np.int64)
                sp[:len(s)] = s
                acc.append(sp)
                off = np.full(ns, 255, np.int64)     # 255 => padded slot
                off[:len(d)] = d[:len(d)] - (c * SLICE + b * 128)
                lo.append(off)
        z = bool(os.environ.get("K_IDX0"))    # timing experiment only
        idxA.append(_wrap_idx16(np.concatenate(la) * 0 if z else np.concatenate(la)))
        idxB.append(_wrap_idx16(np.concatenate(lb) * 0 if z else np.concatenate(lb)))
        doff.append(_slotmajor(np.concatenate(lo), 255, np.float32))

    # --- label edges: 4 groups by (ls-half, ld-half), padded per group ---
    ls = edge_label_index[0].astype(np.int64)
    ld_ = edge_label_index[1].astype(np.int64)
    tls_r = TROW[ls]
    tld_r = TROW[ld_]
    elpc = (EL + 7) // 8                      # 12500 label edges per core
    groups_sz = np.zeros((8, 4), np.int64)
    per_dec = [[None] * 4 for _ in range(8)]
    for c in range(8):
        lo_, hi_ = c * elpc, min((c + 1) * elpc, EL)
        eidx = np.arange(lo_, hi_)
        g = (tls_r[eidx] >= HALF).astype(np.int64) * 2 + (tld_r[eidx] >= HALF)
        for gi in range(4):
            per_dec[c][gi] = eidx[g == gi]
            groups_sz[c, gi] = len(per_dec[c][gi])
    GSZ = [int(-(-groups_sz[:, gi].max() // DEC_TILE) * DEC_TILE)
           for gi in range(4)]
    SL = sum(GSZ)
    tb = np.asarray(trans_bias, np.float32)
    bl2v = float(np.asarray(bl2).reshape(-1)[0])
    lsw, ldw, dbias, slotmap = [], [], [], []
    for c in range(8):
        a_ls = np.zeros(SL, np.int64)
        a_ld = np.zeros(SL, np.int64)
        a_bias = np.zeros(SL, np.float64)
        smap = np.full(SL, -1, np.int64)
        pos = 0
        for gi in range(4):
            e = per_dec[c][gi]
            n = len(e)
            a_ls[pos:pos + n] = tls_r[e] - (HALF if gi >= 2 else 0)
            a_ld[pos:pos + n] = tld_r[e] - (HALF if gi % 2 else 0)
            a_bias[pos:pos + n] = tb[types[ls[e]], types[ld_[e]]]
            smap[pos:pos + n] = e
            pos += GSZ[gi]
        lsw.append(_wrap_idx16(a_ls))
        ldw.append(_wrap_idx16(a_ld))
        dbias.append(_slotmajor(a_bias + bl2v, 0, np.float32))
        slotmap.append(smap)

    iota128 = np.tile(np.arange(128, dtype=np.float32)[None, :], (128, 1))
    ident = np.eye(128, dtype=np.float32)

    in_maps = []
    for c in range(8):
        xcT = np.zeros((COMB, SLICE), np.float32)
        n0 = c * SLICE
        n1 = min((c + 1) * SLICE, N)
        if n1 > n0:
            xcT[:, :n1 - n0] = x_comb[n0:n1].T
        in_maps.append(dict(
            xcT=xcT,
            RHS1=RHS[0], RHS2=RHS[1], RHS3=RHS[2],
            idxA=idxA[c], idxB=idxB[c], doff=doff[c],
            ls_idx=lsw[c], ld_idx=ldw[c], dec_bias=dbias[c],
            iota128=iota128, ident=ident,
            Wl1a=Wl1[:128].astype(bf16), Wl1b=Wl1[128:].astype(bf16),
            Wl2=Wl2.astype(bf16), bl1=bl1.reshape(64, 1).astype(np.float32),
        ))
    cfg = dict(CA=CA, CB=CB, GSZ=GSZ, SL=SL,
               SA=sum(CA) * 128, SB=sum(CB) * 128,
               ST=(sum(CA) + sum(CB)) * 128,
               slotmap=slotmap)
    return in_maps, cfg


# ---------------------------------------------------------------- golden ---
def golden(in_maps, cfg):
    """numpy mirror of the device algorithm (fp32; layout-accurate)."""
    CA, CB = cfg["CA"], cfg["CB"]
    T = None
    for li, (Din, HD, H, D, W_, OFF) in enumerate(LCFG):
        Tn = np.zeros((NP, W_), np.float32)
        slices = []
        for c in range(8):
            im = in_maps[c]
            if li == 0:
                hrow = im["xcT"].T @ im["RHS1"]
            else:
                hrow = PREV[c] @ im[f"RHS{li + 1}"]
            sl = np.zeros((SLICE, W_), np.float32)
            sl[:, :hrow.shape[1]] = hrow
            slices.append(sl)
            Tn[c * SLICE:(c + 1) * SLICE] = sl
        Tp = np.zeros_like(Tn)
        Tp[TROW] = Tn          # permuted chunk-major table layout
        T = Tp.astype(bf16).astype(np.float32)
        slices = [s.astype(bf16).astype(np.float32) for s in slices]
        # edge phase
        PREV = []
        for c in range(8):
            im = in_maps[c]
            ia = _unwrap(im["idxA"], cfg["SA"])
            ib = _unwrap(im["idxB"], cfg["SB"])
            dof = im["doff"].T.reshape(-1)  # slot order
            xl = np.zeros((SLICE, HD), np.float32)
            pa = pb = pt = 0
            for b in range(NB):
                sA, sB = CA[b] * 128, CB[b] * 128
                gidx = np.concatenate([ia[pa:pa + sA],
                                       ib[pb:pb + sB] + HALF])
                pa += sA; pb += sB
                nsl = sA + sB
                G = T[gidx, :]                     # [nsl, W_]
                off = dof[pt:pt + nsl]
                pt += nsl
                S = (off[:, None] == np.arange(128)[None, :]).astype(np.float32)
                als = G[:, OFF:OFF + H]
                # ald per edge via one-hot lookup against the block's rows
                ald_blk = slices[c][b * 128:(b + 1) * 128, OFF + H:OFF + 2 * H]
                ald = S @ ald_blk.astype(bf16).astype(np.float32)
                lg = als + ald
                e = np.maximum(np.exp(lg), np.exp(0.2 * lg))
                e = e.astype(bf16).astype(np.float32)
                msg = (G[:, :HD].reshape(nsl, H, D) * e[:, :, None]
                       ).reshape(nsl, HD).astype(bf16).astype(np.float32)
                num = S.T @ msg                    # [128, HD]
                den = S.T @ e                      # [128, H]
                r = 1.0 / (den + 1e-16)
                xb = (num.reshape(128, H, D) * r[:, :, None]).reshape(128, HD)
                if li < 2:
                    xb = np.maximum(xb, 0) - 1 + np.exp(np.minimum(xb, 0))
                xl[b * 128:(b + 1) * 128] = xb
            PREV.append(xl)
    # decode
    TDn = np.zeros((NP, TDEC_W), np.float32)
    for c in range(8):
        TDn[c * SLICE:(c + 1) * SLICE] = PREV[c].astype(bf16)
    TD = np.zeros_like(TDn)
    TD[TROW] = TDn             # permuted chunk-major layout
    TD = TD.astype(bf16).astype(np.float32)
    scores = []
    for c in range(8):
        im = in_maps[c]
        lsv = _unwrap(im["ls_idx"], cfg["SL"])
        ldv = _unwrap(im["ld_idx"], cfg["SL"])
        base_ls = np.zeros(cfg["SL"], np.int64)
        base_ld = np.zeros(cfg["SL"], np.int64)
        pos = 0
        for gi in range(4):
            base_ls[pos:pos + cfg["GSZ"][gi]] = HALF if gi >= 2 else 0
            base_ld[pos:pos + cfg["GSZ"][gi]] = HALF if gi % 2 else 0
            pos += cfg["GSZ"][gi]
        zl = TD[lsv + base_ls]
        zr = TD[ldv + base_ld]
        W1a = im["Wl1a"].astype(np.float32)
        W1b = im["Wl1b"].astype(np.float32)
        h = np.maximum(zl @ W1a + zr @ W1b + im["bl1"].T, 0).astype(bf16).astype(np.float32)
        base = h @ im["Wl2"].astype(np.float32)
        bias = im["dec_bias"].T.reshape(-1)
        scores.append(base[:, 0] + bias)
    out = np.zeros((EL, 1), np.float32)
    for c in range(8):
        m = cfg["slotmap"][c] >= 0
        out[cfg["slotmap"][c][m], 0] = scores[c][m]
    return out


def _unwrap(w, n):
    return w[:16, :].T.reshape(-1)[:n].astype(np.int64)


# ----------------------------------------------------------------- device ---
def build(cfg):
    import concourse.bacc as bacc
    import concourse.mybir as mybir
    from concourse.tile import TileContext
    dt = mybir.dt
    F = mybir.ActivationFunctionType
    A = mybir.AluOpType
    CA, CB, SL = cfg["CA"], cfg["CB"], cfg["SL"]
    SA, SB, ST = cfg["SA"], cfg["SB"], cfg["ST"]

    GO = bool(os.environ.get("K_GATHERONLY"))   # timing experiment only
    scratch = int(os.environ.get("K_SCRATCH", "32768"))
    nc = bacc.Bacc(num_devices=8, dynamic_dma_scratch_size=scratch)
    # max 128-chunks per dma_gather call. The HW SWDGE descriptor ring
    # limits per-call size; 6 (768 idx) is proven stable at 32KB scratch.
    GMAX = int(os.environ.get("K_GMAX", "6"))

    def gat(out_ap, in_ap, idx_tile, col0, nchunk, elem, **kw):
        for s0 in range(0, nchunk, GMAX):
            s1 = min(s0 + GMAX, nchunk)
            nc.gpsimd.dma_gather(
                out_ap[:, s0:s1, :], in_ap,
                idx_tile[:, col0 + s0 * 8: col0 + s1 * 8],
                (s1 - s0) * 128, (s1 - s0) * 128, elem, **kw)

    inp = {}
    for name, shape, d in [
        ("xcT", [COMB, SLICE], dt.float32),
        ("RHS1", [48, 264], dt.float32),
        ("RHS2", [256, 260], dt.float32),
        ("RHS3", [256, 130], dt.float32),
        ("idxA", [16, SA // 16], dt.int16),
        ("idxB", [16, SB // 16], dt.int16),
        ("doff", [128, ST // 128], dt.float32),
        ("ls_idx", [16, SL // 16], dt.int16),
        ("ld_idx", [16, SL // 16], dt.int16),
        ("dec_bias", [128, SL // 128], dt.float32),
        ("iota128", [128, 128], dt.float32),
        ("ident", [128, 128], dt.float32),
        ("Wl1a", [128, 64], dt.bfloat16),
        ("Wl1b", [128, 64], dt.bfloat16),
        ("Wl2", [64, 1], dt.bfloat16),
        ("bl1", [64, 1], dt.float32),
    ]:
        inp[name] = nc.dram_tensor(name, shape, d, kind="ExternalInput")
    score_out = nc.dram_tensor("score", [SL, 1], dt.float32, kind="ExternalOutput")

    sl_t = [nc.dram_tensor(f"slice{l}", [SLICE, LCFG[l][4]], dt.bfloat16,
                           kind="Internal") for l in range(3)]
    sl_d = nc.dram_tensor("sliceD", [SLICE, TDEC_W], dt.bfloat16, kind="Internal")
    T_t = [nc.dram_tensor(f"T{l}", [NP, LCFG[l][4]], dt.bfloat16,
                          kind="Internal", addr_space="Shared") for l in range(3)]
    T_d = nc.dram_tensor("TD", [NP, TDEC_W], dt.bfloat16,
                         kind="Internal", addr_space="Shared")

    def chunked_allgather(sl_tensor, T_tensor, width, q):
        if os.environ.get("K_SKIPCOLL"):      # timing experiment only
            return
        r0, r1 = CHUNK_R0[q], CHUNK_R0[q] + CHUNK_R[q]
        o0 = CHUNK_O[q]
        nc.gpsimd.collective_compute(
            "AllGather", mybir.AluOpType.bypass,
            ins=[sl_tensor[r0:r1, :]],
            outs=[T_tensor[o0:o0 + 8 * CHUNK_R[q], :]],
            replica_groups=[list(range(8))])

    with TileContext(nc, num_cores=8) as tc:
        with tc.tile_pool(name="const", bufs=1) as cpool, \
             tc.tile_pool(name="work", bufs=2) as wpool, \
             tc.tile_pool(name="psum", bufs=2, space="PSUM") as ppool, \
             tc.tile_pool(name="psum1", bufs=1, space="PSUM") as ppool1:
            # ---- resident constants / indices ----
            def load(name, shape, d):
                t = cpool.tile(shape, d, tag=name)
                nc.sync.dma_start(t[:], inp[name][:])
                return t

            def load_idx(name, cols):
                """int16 idx [16, cols] replicated into [128, cols]."""
                t = cpool.tile([128, cols], dt.int16, tag=name)
                for g in range(8):
                    nc.sync.dma_start(t[16 * g:16 * g + 16, :], inp[name][:])
                return t
            idxA = load_idx("idxA", SA // 16)
            idxB = load_idx("idxB", SB // 16)
            doff = load("doff", [128, ST // 128], dt.float32)
            # per-layer resident [128, NB, H] dst-fold values (tiny): written
            # by the prologue / node phase, read by the next layer's ald matmul
            aldt = []
            for l in range(3):
                ald_l = cpool.tile([128, NB, LCFG[l][2]], dt.bfloat16,
                                   tag=f"ald{l}", name=f"ald{l}")
                aldt.append(ald_l)
            iota = load("iota128", [128, 128], dt.float32)
            ident = load("ident", [128, 128], dt.float32)
            identb = cpool.tile([128, 128], dt.bfloat16, tag="identb")
            nc.vector.tensor_copy(identb[:], ident[:])
            # decoder constants loaded up front so their DMAs overlap the
            # GAT layers instead of serializing into the decode tail
            lsi = load_idx("ls_idx", SL // 16)
            ldi = load_idx("ld_idx", SL // 16)
            dbias = load("dec_bias", [128, SL // 128], dt.float32)
            W1a = load("Wl1a", [128, 64], dt.bfloat16)
            W1b = load("Wl1b", [128, 64], dt.bfloat16)
            W2d = load("Wl2", [64, 1], dt.bfloat16)
            bl1 = load("bl1", [64, 1], dt.float32)
            RHSs = [load("RHS1", [48, 264], dt.float32)]
            for l, w in ((2, 260), (3, 130)):
                t = cpool.tile([128, 2, w], dt.float32, tag=f"RHS{l}")
                nc.sync.dma_start(
                    t[:], inp[f"RHS{l}"][:].rearrange("(k p) w -> p k w", p=128))
                RHSs.append(t)

            # ---- prologue: T1 rows = x_comb @ RHS1 ----
            W0 = LCFG[0][4]
            done_blocks = 0
            for q in range(NCHUNK):
                for b in range(done_blocks, done_blocks + CHUNK_BLOCKS[q]):
                    if GO:
                        continue
                    xc = wpool.tile([COMB, 128], dt.float32, tag="xc")
                    nc.sync.dma_start(xc[:], inp["xcT"][:, b * 128:(b + 1) * 128])
                    pn = ppool1.tile([128, 264], dt.float32, tag="pn")
                    nc.tensor.matmul(pn[:, 0:264],
                                     xc[:],
                                     RHSs[0][:], start=True, stop=True)
                    row = wpool.tile([128, W0], dt.bfloat16, tag="row")
                    nc.vector.tensor_copy(row[:, 0:264], pn[:, 0:264])
                    nc.vector.memset(row[:, 264:W0], 0)
                    nc.vector.tensor_copy(aldt[0][:, b, :], pn[:, 260:264])
                    nc.sync.dma_start(sl_t[0][b * 128:(b + 1) * 128, :],
                                      row[:, 0:W0])
                done_blocks += CHUNK_BLOCKS[q]
                chunked_allgather(sl_t[0], T_t[0], W0, q)

            # ---- three GAT layers ----
            for li, (Din, HD, H, D, W_, OFF) in enumerate(LCFG):
                RW = HD + H
                ALD0 = OFF + H - (W_ - 128)   # ald offset inside GD window
                pa = pb = pt_ = 0
                done_blocks = 0
                for q in range(NCHUNK):
                  for b in range(done_blocks, done_blocks + CHUNK_BLOCKS[q]):
                    cA, cB = CA[b], CB[b]
                    C = cA + cB
                    G = wpool.tile([128, C, W_], dt.bfloat16, tag="G")
                    if os.environ.get("K_GELEM128"):  # timing experiment only
                        G128 = wpool.tile([128, C, 128], dt.bfloat16, tag="G128")
                        gat(G128[:, :cA, :].rearrange("p c e -> p c e"),
                            T_t[li][:, 0:128], idxA[:], pa // 16, cA,
                            128, elem_step=W_)
                        if cB:
                            gat(G128[:, cA:C, :].rearrange("p c e -> p c e"),
                                T_t[li][HALF:, 0:128], idxB[:], pb // 16, cB,
                                128, elem_step=W_)
                    else:
                        gat(G, T_t[li][:, :], idxA[:], pa // 16, cA, W_)
                        if cB:
                            gat(G[:, cA:C, :].rearrange("p c e -> p c e"),
                                T_t[li][HALF:, :], idxB[:], pb // 16, cB, W_)
                    if GO:
                        pa += cA * 128
                        pb += cB * 128
                        pt_ += C * 128
                        continue
                    S = wpool.tile([128, C, 128], dt.bfloat16, tag="S")
                    nc.vector.tensor_tensor(
                        S[:],
                        doff[:, pt_ // 128: pt_ // 128 + C].unsqueeze(-1)
                            .broadcast_to([128, C, 128]),
                        iota[:].unsqueeze(1).broadcast_to([128, C, 128]),
                        A.is_equal)
                    # ald per edge slot: transpose each S chunk on TensorE,
                    # then one-hot matmul against the block's ald rows
                    ST = wpool.tile([128, C, 128], dt.bfloat16, tag="ST")
                    pald = ppool1.tile([128, C, H], dt.float32, tag="pald")
                    for ch in range(C):
                        ptp = ppool1.tile([128, 128], dt.bfloat16, tag="ptb")
                        nc.tensor.transpose(ptp[:], S[:, ch, :], identb[:])
                        nc.vector.tensor_copy(ST[:, ch, :], ptp[:])
                        nc.tensor.matmul(pald[:, ch, :], ST[:, ch, :],
                                         aldt[li][:, b, :],
                                         start=True, stop=True)
                    lg = wpool.tile([128, C, H], dt.float32, tag="lg")
                    nc.vector.tensor_tensor(
                        lg[:], G[:, :, OFF:OFF + H], pald[:], A.add)
                    # e = exp(leaky_relu(lg)) = max(exp(lg), exp(0.2*lg))
                    e1 = wpool.tile([128, C, H], dt.float32, tag="e1")
                    nc.scalar.activation(e1[:], lg[:], F.Exp)
                    e2 = wpool.tile([128, C, H], dt.float32, tag="e2")
                    nc.scalar.activation(e2[:], lg[:], F.Exp, scale=0.2)
                    RT = wpool.tile([128, C, RW], dt.bfloat16, tag="RT")
                    nc.vector.tensor_tensor(RT[:, :, HD:HD + H], e1[:], e2[:],
                                            A.max)
                    nc.vector.tensor_tensor(
                        RT[:, :, 0:HD].rearrange("p c (h d) -> p c h d", h=H),
                        G[:, :, 0:HD].rearrange("p c (h d) -> p c h d", h=H),
                        RT[:, :, HD:HD + H].unsqueeze(-1)
                            .broadcast_to([128, C, H, D]),
                        A.mult)
                    pe = ppool.tile([128, RW], dt.float32, tag="pe")
                    for ch in range(C):
                        nc.tensor.matmul(pe[:, 0:RW], S[:, ch, :], RT[:, ch, :],
                                         start=(ch == 0), stop=(ch == C - 1))
                    pa += cA * 128
                    pb += cB * 128
                    pt_ += C * 128
                    # ---- finalize + node phase ----
                    den = wpool.tile([128, H], dt.float32, tag="den")
                    nc.vector.tensor_scalar_add(den[:], pe[:, HD:HD + H], 1e-16)
                    rec = wpool.tile([128, H], dt.float32, tag="rec")
                    nc.vector.reciprocal(rec[:], den[:])
                    xo = wpool.tile([128, HD], dt.float32, tag="xo")
                    nc.vector.tensor_tensor(
                        xo[:].rearrange("p (h d) -> p h d", h=H),
                        pe[:, 0:HD].rearrange("p (h d) -> p h d", h=H),
                        rec[:].unsqueeze(-1).broadcast_to([128, H, D]),
                        A.mult)
                    if li < 2:
                        m = wpool.tile([128, HD], dt.float32, tag="melu")
                        nc.vector.tensor_scalar_min(m[:], xo[:], 0.0)
                        e3 = wpool.tile([128, HD], dt.float32, tag="e3")
                        nc.scalar.activation(e3[:], m[:], F.Exp)
                        # xo = (max(xo,0) - 1) + exp(min(xo,0))
                        nc.vector.tensor_scalar(xo[:], xo[:], 0.0, -1.0,
                                                A.max, A.add)
                        nc.vector.tensor_tensor(xo[:], xo[:], e3[:], A.add)
                        # node phase: T_{l+1} row = [x @ W' | folds]
                        NDin, NHD, NH, ND, NW_, _ = LCFG[li + 1]
                        NW = NHD + 2 * NH
                        xT = wpool.tile([128, 2, 128], dt.float32, tag="xT")
                        for kc in range(2):
                            ptp = ppool1.tile([128, 128], dt.float32, tag="pt")
                            nc.tensor.transpose(
                                ptp[:], xo[:, kc * 128:(kc + 1) * 128], ident[:])
                            nc.vector.tensor_copy(xT[:, kc, :], ptp[:])
                        pn = ppool1.tile([128, 264], dt.float32, tag="pn")
                        for kc in range(2):
                            nc.tensor.matmul(pn[:, 0:NW], xT[:, kc, :],
                                             RHSs[li + 1][:, kc, :],
                                             start=(kc == 0), stop=(kc == 1))
                        row = wpool.tile([128, NW_], dt.bfloat16, tag="nrow")
                        nc.vector.tensor_copy(row[:, 0:NW], pn[:, 0:NW])
                        if NW < NW_:
                            nc.vector.memset(row[:, NW:NW_], 0)
                        nc.vector.tensor_copy(
                            aldt[li + 1][:, b, :],
                            pn[:, NHD + NH:NHD + 2 * NH])
                        nc.sync.dma_start(
                            sl_t[li + 1][b * 128:(b + 1) * 128, :],
                            row[:, 0:NW_])
                    else:
                        rowd = wpool.tile([128, TDEC_W], dt.bfloat16, tag="rowd")
                        nc.vector.tensor_copy(rowd[:], xo[:])
                        nc.sync.dma_start(
                            sl_d[b * 128:(b + 1) * 128, :], rowd[:])
                  done_blocks += CHUNK_BLOCKS[q]
                  if li < 2:
                      chunked_allgather(sl_t[li + 1], T_t[li + 1],
                                        LCFG[li + 1][4], q)
                  else:
                      chunked_allgather(sl_d, T_d, TDEC_W, q)

            # ---- decoder ----
            score_sb = cpool.tile([128, SL // 128], dt.float32, tag="score")
            if GO:
                nc.vector.memset(score_sb[:], 0)
            pos = 0
            for gi in range(4):
                gls, gld = (HALF if gi >= 2 else 0), (HALF if gi % 2 else 0)
                t0 = pos
                while t0 < pos + cfg["GSZ"][gi]:
                    nidx = min(DEC_GATHER, pos + cfg["GSZ"][gi] - t0)
                    zl = wpool.tile([128, 1, nidx], dt.bfloat16, tag=f"zl{nidx}")
                    nc.gpsimd.dma_gather(
                        zl[:], T_d[gls:, :], lsi[:, t0 // 16:(t0 + nidx) // 16],
                        nidx, nidx, TDEC_W, transpose=True)
                    zr = wpool.tile([128, 1, nidx], dt.bfloat16, tag=f"zr{nidx}")
                    nc.gpsimd.dma_gather(
                        zr[:], T_d[gld:, :], ldi[:, t0 // 16:(t0 + nidx) // 16],
                        nidx, nidx, TDEC_W, transpose=True)
                    for s0 in range(0, nidx, DEC_TILE):
                        if GO:
                            continue
                        ph = ppool1.tile([64, DEC_TILE], dt.float32, tag="ph")
                        nc.tensor.matmul(ph[:], W1a[:], zl[:, 0, s0:s0 + DEC_TILE],
                                         start=True, stop=False)
                        nc.tensor.matmul(ph[:], W1b[:], zr[:, 0, s0:s0 + DEC_TILE],
                                         start=False, stop=True)
                        hd = wpool.tile([64, DEC_TILE], dt.bfloat16, tag="hd")
                        nc.scalar.activation(hd[:], ph[:], F.Relu, bias=bl1[:])
                        for sub in range(DEC_TILE // 128):
                            pss = ppool1.tile([128, 1], dt.float32, tag="pss")
                            nc.tensor.matmul(
                                pss[:], hd[:, sub * 128:(sub + 1) * 128],
                                W2d[:], start=True, stop=True)
                            col = (t0 + s0) // 128 + sub
                            nc.vector.tensor_tensor(
                                score_sb[:, col:col + 1], pss[:],
                                dbias[:, col:col + 1], A.add)
                    t0 += nidx
                pos += cfg["GSZ"][gi]
            nc.sync.dma_start(
                score_out[:].rearrange("(c p) o -> p (c o)", p=128), score_sb[:])
    nc.finalize()
    return nc


def kernel(**inputs):
    inputs = {k: np.asarray(v) for k, v in inputs.items()}
    in_maps, cfg = prep(**inputs)
    nc = build(cfg)
    from concourse.bass_utils import run_bass_kernel_spmd
    res = run_bass_kernel_spmd(nc, in_maps, core_ids=list(range(8)))
    out = np.zeros((EL, 1), np.float32)
    for c in range(8):
        sc = res.results[c]["score"][:, 0]
        m = cfg["slotmap"][c] >= 0
        out[cfg["slotmap"][c][m], 0] = sc[m]
    return out
